# revision 52
# baseline (speedup 1.0000x reference)
"""Bass/Trainium2 kernel for nn_AttentionPooling2 (segment_reduce).

Math (per batch b):
    scores = gelu(LN(doc_state @ W1 + b1) * gamma + beta) @ W2 + b2      # (S,)
    logits = M * scores + (1-M) * (-1e4);  attn = softmax_S(logits)
    pooled = einsum('ns,ns,sd->nd', M, attn, doc_state)

Because M is binary and exp(-1e4 - max) underflows to exactly 0 in fp32,
the reference collapses to
    pooled[n] = (M[n] * e) @ X / (M[n] @ e),   e = exp(scores)
(max-subtraction and b2 cancel in the ratio).

Per-core plan (pure data-parallel, batch b -> core b), 13308ns on the
CoreSim cost model vs the 19570ns prior kernel:
  * X ships TWICE from the host: xT [D, S] feeds the scorer matmuls
    directly (no device transposes / PSUM round trips), x_pad [S, D+2]
    feeds the pooled matmul.  All matmul operands are float32r.
  * h = X @ W1 on PE; an extra host-precomputed rhs column pair
    (rowsum(W1)/D) makes PE emit per-token means for free.
  * LN variance: DVE bn_stats for 6 tiles + ACT Square-with-accumulator
    for the last tile of each half (GPSIMD cannot touch PSUM, nothing
    may read two PSUM operands, and accum_out exists only on DVE/ACT).
  * rstd = 1/sqrt(var+eps) via division-free rsqrt-Newton (seed
    1.5-vh, 2 iterations, ~1e-6 rel) on the Pool engine, whose [P,4]
    ops cost ~3ns; the act table is loaded ONCE (gelu set, t~300, warm
    gelu pins the set choice) and never switched.
  * gelu fuses LN via per-partition scale=rstd / bias=-mean*rstd; score
    dots ride DVE (the only accumulator with spare throughput).
  * e = exp(s) = (e^(s/8))^8 via a degree-7 Taylor Horner chain plus
    three squarings, entirely on Pool (~50ns, rel err ~2e-5) -- no ACT
    tanh and no DVE reciprocal on the score->pooled critical chain.
  * mask scaling all on Pool (DVE's queue is score-saturated); pooled
    num on PE with a separate early-finishing denominator chain (ones
    columns); single [P,256] normalize + single output DMA (a split
    pair serializes on the shared po tile and pays the HWDGE stagger).

Hard-won scheduling rules encoded below (Bacc TileScheduler is greedy
by readiness with one coarse sem wait per instruction):
  * distinct engine queues = the parallelism; a queue's DMAs serialize
    and their TRANSFER completions gate later compute on that queue, so
    Pool carries only the two leading xt chunks and ACT carries none.
  * even disjoint-region accesses of one tile serialize across engines,
    so every cross-stage tensor is split per half / per lane (ph pairs,
    s_col, mts, out halves, stats/mv).
  * emission order seeds the schedule: blocks are emitted in intended
    time order, sq7+NewtonB after g1, WAW gates pin NewtonB after
    NewtonA on Pool.
  * PE p-state decays when idle; 6 tiny gated "poke" matmuls keep the
    clock ramped so the pooled chain runs at 107ns/matmul.
"""

import numpy as np

B, S, N, D = 8, 1024, 128, 256
P = 128          # partitions
ST = S // P      # 8 token tiles
DC = D // P      # 2 contraction chunks
LN_EPS = 1e-5

_CACHE = {}


def _runs(cols):
    """Group sorted column indices into contiguous [lo, hi) runs."""
    out = []
    for c in cols:
        if out and out[-1][1] == c:
            out[-1][1] = c + 1
        else:
            out.append([c, c + 1])
    return [(lo, hi, None) for lo, hi in out]


def _build(fast_ln: bool):
    from contextlib import ExitStack

    import concourse.bass as bass
    import concourse.tile as tile
    from concourse import bacc, mybir

    f32 = mybir.dt.float32
    f32r = mybir.dt.float32r
    AF = mybir.ActivationFunctionType
    OP = mybir.AluOpType

    nc = bacc.Bacc("TRN2")
    xt = nc.dram_tensor("xt", [D, S], f32r, kind="ExternalInput")
    xp = nc.dram_tensor("xp", [S, D + 2], f32r, kind="ExternalInput")
    mt = nc.dram_tensor("mt", [S, N], f32r, kind="ExternalInput")
    # [c0|c1] = W1 chunks (+ mean col 256), [2] = host-broadcast W2 row
    w1m = nc.dram_tensor("w1m", [P, 3, D + 2], f32r, kind="ExternalInput")
    if not fast_ln:
        b1d = nc.dram_tensor("b1", [1, D], f32, kind="ExternalInput")
        gmd = nc.dram_tensor("gamma", [1, D], f32, kind="ExternalInput")
        btd = nc.dram_tensor("beta", [1, D], f32, kind="ExternalInput")
    out = nc.dram_tensor("out", [N, D], f32, kind="ExternalOutput")

    xt_re = xt.rearrange("(c p) s -> p c s", p=P)        # [128, 2, 1024]
    xp_re = xp.rearrange("(t p) j -> p t j", p=P)        # [128, 8, 258]
    mt_re = mt.rearrange("(t p) n -> p t n", p=P)        # [128, 8, 128]

    def bcast(handle):  # [1, D] dram -> [[0,P],[1,D]] broadcast AP
        return bass.AP(handle, 0, [[0, P], [1, D]])

    with tile.TileContext(nc) as tc, ExitStack() as ctx:
        consts = ctx.enter_context(tc.tile_pool(name="consts", bufs=1))
        big = ctx.enter_context(tc.tile_pool(name="big", bufs=1))
        gelu_p = ctx.enter_context(tc.tile_pool(name="gelu", bufs=3))
        scr_p = ctx.enter_context(tc.tile_pool(name="scr", bufs=2))
        sq_p = ctx.enter_context(tc.tile_pool(name="sq", bufs=2))
        ps_h = ctx.enter_context(tc.tile_pool(name="ps_h", bufs=1, space="PSUM"))
        ps_o = ctx.enter_context(tc.tile_pool(name="ps_o", bufs=1, space="PSUM"))
        sq_ps = ctx.enter_context(tc.tile_pool(name="sq_ps", bufs=2, space="PSUM"))

        xt_sb = big.tile([P, DC, S], f32r)
        xp_sb = big.tile([P, ST, D + 2], f32r)
        mt_sb = big.tile([P, ST, N], f32r)
        w1m_sb = big.tile([P, 3, D + 2], f32r)
        w2_sb = w1m_sb.bitcast(f32)[:, 2, 0:D]

        # DMA spread: only SP/ACT have HWDGE (a shared ~630ns/DMA mutex;
        # transfers then run in parallel) and Pool has SWDGE (~1.3us of
        # Pool-engine descriptor gen, paid while Pool is idle early).
        # w1m + the leading xT chunk gate the first matmuls, so they get
        # the first HWDGE slots; x_pad is only needed by the pooled
        # matmuls (~t+6us) so it rides last; the mask rides Pool.
        # Warm gelu as the very first ACT op: the act-table pass picks
        # the set for the FIRST table-needing activation -- a Square
        # would select the exp set and force a 1283ns reload before the
        # gelus.  Emitted before everything so the load lands at t~300.
        warm = consts.tile([P, 1], f32)
        nc.vector.memset(warm, 0.0)
        g_warm = consts.tile([1, 1], f32)
        nc.scalar.activation(out=g_warm, in_=warm[0:1, :], func=AF.Gelu)

        # Two parallel DMA lanes: the SP HWDGE queue and the Pool SWDGE
        # queue (a queue's transfers serialize; distinct queues overlap).
        # ACT carries NO dma so its queue opens with the single act-table
        # load at t~300 -- an ACT-queue DMA would push the load (and the
        # first Square) past ~4us.  W1 chunks + xt go first (they gate
        # the h matmuls; Pool gets xt[0:384] so tiles 0-2 land first),
        # W2 / x_pad / mask follow (needed only from the score phase on).
        # Pool carries ONLY the two leading xt chunks: a Pool compute op
        # emitted after a Pool-queue DMA waits (via the in-order counting
        # sem) for that DMA's TRANSFER to complete, so any late Pool DMA
        # would stall the Newton chains.  Everything else rides SP in
        # deadline order (w2 before mask/x_pad: scores need it ~7us).
        nc.sync.dma_start(out=w1m_sb[:, 0:2, :], in_=w1m[:, 0:2, :])
        nc.gpsimd.dma_start(out=xt_sb[:, :, 0:256], in_=xt_re[:, :, 0:256])
        nc.gpsimd.dma_start(out=xt_sb[:, :, 256:512], in_=xt_re[:, :, 256:512])
        nc.sync.dma_start(out=w1m_sb[:, 2:3, :], in_=w1m[:, 2:3, :])
        nc.sync.dma_start(out=xt_sb[:, :, 512:1024], in_=xt_re[:, :, 512:1024])
        nc.sync.dma_start(out=mt_sb, in_=mt_re)
        nc.sync.dma_start(out=xp_sb[:, 0:4, :], in_=xp_re[:, 0:4, :])
        nc.sync.dma_start(out=xp_sb[:, 4:8, :], in_=xp_re[:, 4:8, :])
        if not fast_ln:
            b1_sb = consts.tile([P, D], f32)
            gm_sb = consts.tile([P, D], f32)
            bt_sb = consts.tile([P, D], f32)
            nc.gpsimd.dma_start(out=b1_sb, in_=bcast(b1d))
            nc.gpsimd.dma_start(out=gm_sb, in_=bcast(gmd))
            nc.gpsimd.dma_start(out=bt_sb, in_=bcast(btd))

        # ones column pair for the pooled denominator chain (f32r can't
        # be memset directly)
        ones_f = consts.tile([P, 2], f32)
        nc.vector.memset(ones_f, 1.0)
        ones_r = consts.tile([P, 2], f32r)
        nc.vector.tensor_copy(out=ones_r, in_=ones_f)

        # Cross-engine deps are tracked at TILE granularity: one shared
        # [P, 8] tile would make e.g. halfB writers wait on halfA readers.
        # Everything that two pipeline stages touch concurrently is split
        # per half (index [half]).
        HALF = 4

        def half_tiles(name):
            return [consts.tile([P, HALF], f32, name=f"{name}{h}",
                                tag=f"{name}{h}")
                    for h in range(2)]

        ssq = half_tiles("ssq")      # sum of h^2 per tile
        mu = half_tiles("mu")        # mean per tile
        rstd = half_tiles("rstd")
        nmr = half_tiles("nmr")      # -mean * rstd
        s_col = half_tiles("s_col")  # scores
        e_col = half_tiles("e_col")  # exp(scores)
        th = half_tiles("th")
        e_den = half_tiles("e_den")
        nt1 = half_tiles("nt1")      # newton temps
        nt2 = half_tiles("nt2")
        ns_ = half_tiles("ns_")

        # even disjoint-region accesses of a shared tile serialize across
        # engines, so h tiles are paired BY STAT LANE: (0,1),(2,6),(4,5)
        # hold DVE-bn tiles and (3,7) the ACT-square tiles -- no PSUM
        # tile is read by both stat lanes, each pair packs one 2KB bank.
        ph_pair = [ps_h.tile([P, 2, D], f32, name=f"php{i}", tag=f"php{i}")
                   for i in range(4)]
        _PH_SLOT = {0: (0, 0), 1: (0, 1), 2: (1, 0), 6: (1, 1),
                    4: (2, 0), 5: (2, 1), 3: (3, 0), 7: (3, 1)}

        def ph_t(t):
            p, i = _PH_SLOT[t]
            return ph_pair[p][:, i, :]
        po = ps_o.tile([P, 512], f32)        # 1 bank: pooled numerator
        # den + mean columns live in their own bank: a PSUM zone allows
        # only one pending accumulation group, and den's group is open
        # concurrently with num's
        pd = ps_o.tile([P, 512], f32, tag="pd")

        # stat-lane split: only DVE/ACT may touch PSUM (GPSIMD cannot at
        # all, and no op may read two PSUM operands).  DVE tiles use
        # bn_stats (one PSUM read, 459ns); ACT tiles use Square+accum
        # (544ns).  ACT is the saturated engine (squares then 8 gelus
        # back-to-back in its in-order queue), so it gets only one tile
        # per half -- chosen as the LAST tile of each half so NewtonX
        # fires as soon as that square lands.
        BN_TILES = (0, 1, 2, 4, 5, 6)  # DVE bn_stats lane
        SQ_TILES = (3, 7)              # ACT Square+accum lane
        # bn stats/aggr outputs per half (shared tiles serialize the
        # Newton chain behind the LAST aggr writer otherwise)
        bn_local = {t: i for i, t in
                    enumerate([u for u in BN_TILES if u < 4])}
        bn_local.update({t: i for i, t in
                         enumerate([u for u in BN_TILES if u >= 4])})
        NBN = len([u for u in BN_TILES if u < 4])
        stats_h = [consts.tile([P, NBN, 6], f32, name=f"stats{h}",
                               tag=f"stats{h}") for h in range(2)]
        mv_h = [consts.tile([P, NBN, 2], f32, name=f"mv{h}", tag=f"mv{h}")
                for h in range(2)]

        def emit_bn(t):
            i = bn_local[t]
            sth = stats_h[t // HALF]
            nc.vector.bn_stats(out=sth[:, i, :], in_=ph_t(t))
            nc.vector.bn_aggr(out=mv_h[t // HALF][:, i, :],
                              in_=sth[:, i, :])

        def emit_sq(t):
            # Pool has no PSUM access and no accumulator; the Scalar
            # engine's Square activation + accumulator does the whole
            # sum-of-squares in one PSUM read during ACT's pre-gelu idle.
            # Output goes to a PSUM scratch bank (172-cycle access vs 222
            # for SBUF; the value is dead).
            sq_s = sq_ps.tile([P, D], f32, tag="sqs")
            nc.scalar.activation(out=sq_s, in_=ph_t(t),
                                 func=AF.Square,
                                 accum_out=ssq[t // HALF][:, t % HALF:t % HALF + 1])

        def emit_mms(t):
            ts_ = slice(P * t, P * (t + 1))
            for c in range(DC):
                nc.tensor.matmul(ph_t(t),
                                 lhsT=xt_sb[:, c, ts_],
                                 rhs=w1m_sb[:, c, 0:D],
                                 start=(c == 0), stop=(c == DC - 1))
            if t in SQ_TILES:
                # fp32r matmuls reject 1-wide outputs (s3d3 restrictions);
                # use a 2-wide column pair (second col is host zero-pad)
                for c in range(DC):
                    nc.tensor.matmul(pd[:, 264 + 2 * t:266 + 2 * t],
                                     lhsT=xt_sb[:, c, ts_],
                                     rhs=w1m_sb[:, c, D:D + 2],
                                     start=(c == 0), stop=(c == DC - 1))
            if not fast_ln:
                nc.vector.tensor_tensor(out=ph_t(t),
                                        in0=ph_t(t),
                                        in1=b1_sb, op=OP.add)

        # Emission order seeds the greedy scheduler: a half's matmuls,
        # then its stats lanes (DVE bn + ACT squares), then the next half.
        def emit_mu(half):
            # mean assembly (DVE: the only PSUM-capable engine with slack)
            # emitted right after the half's stats so it isn't queued
            # behind the other half's bn chain on DVE
            h0 = 4 * half
            bn_cols = [t for t in range(h0, h0 + 4) if t in BN_TILES]
            sq_cols = [t for t in range(h0, h0 + 4) if t in SQ_TILES]
            muh = mu[half]
            for lo, hi, src in _runs(sq_cols):
                nc.vector.tensor_copy(
                    out=muh[:, lo - h0:hi - h0],
                    in_=pd.bitcast(f32)[:, 264 + 2 * lo:264 + 2 * hi:2])
            for lo, hi, src in _runs(bn_cols):
                li = bn_local[lo]
                nc.vector.tensor_copy(
                    out=muh[:, lo - h0:hi - h0],
                    in_=mv_h[half][:, li:li + (hi - lo), 0])
            if not fast_ln:
                for lo, hi, src in _runs(sq_cols):
                    nc.vector.tensor_scalar_add(
                        out=muh[:, lo - h0:hi - h0],
                        in0=muh[:, lo - h0:hi - h0],
                        scalar1=w1m_sb.bitcast(f32)[:, 2, D:D + 1])

        for t in range(4):
            emit_mms(t)
        for t in range(4):
            emit_sq(t) if t in SQ_TILES else emit_bn(t)
        emit_mu(0)
        for t in range(4, ST):
            emit_mms(t)
        for t in range(4, ST):
            if t not in SQ_TILES:
                emit_bn(t)
        emit_mu(1)

        # per-half vh assembly + rsqrt-Newton (Pool)
        def emit_newton(half):
            h0 = 4 * half
            bn_cols = [t for t in range(h0, h0 + 4) if t in BN_TILES]
            sq_cols = [t for t in range(h0, h0 + 4) if t in SQ_TILES]
            muh, ssqh = mu[half], ssq[half]
            t1h, t2h, nsh = nt1[half], nt2[half], ns_[half]
            rsh, nmh = rstd[half], nmr[half]
            # vh = 0.5*(var+eps).  Both halves run division-free
            # rsqrt-Newton on the Pool engine: Pool [P,4] ops cost ~3ns
            # in the model (vs ~65ns DVE) and Pool is idle here, so the
            # chain costs ~50ns and leaves DVE free for bn_stats/scores.
            eng = nc.gpsimd
            if half == 1:
                # WAW gates: halfB's temps are first written by reads of
                # halfA's rstd, pinning the whole B chain after A in the
                # in-order Pool stream (the scheduler otherwise interleaves
                # a sq7-gated B op ahead of A's tail, stalling gelu0)
                eng.tensor_scalar_mul(out=t1h, in0=rstd[0], scalar1=1.0)
                eng.tensor_scalar_mul(out=t2h, in0=rstd[0], scalar1=1.0)

            for lo, hi, src in _runs(bn_cols):
                li = bn_local[lo]
                eng.tensor_scalar(
                    out=t2h[:, lo - h0:hi - h0],
                    in0=mv_h[half][:, li:li + (hi - lo), 1],
                    scalar1=0.5, scalar2=LN_EPS * 0.5,
                    op0=OP.mult, op1=OP.add)
            for lo, hi, src in _runs(sq_cols):
                l, h = lo - h0, hi - h0
                # scalar_tensor_tensor has no Pool encoding; compose from
                # tensor_tensor + dual-op tensor_scalar instead
                eng.tensor_tensor(out=t1h[:, l:h], in0=muh[:, l:h],
                                  in1=muh[:, l:h], op=OP.mult)
                eng.tensor_scalar(out=t1h[:, l:h], in0=t1h[:, l:h],
                                  scalar1=-0.5, scalar2=LN_EPS * 0.5,
                                  op0=OP.mult, op1=OP.add)
                eng.tensor_scalar_mul(out=t2h[:, l:h],
                                      in0=ssqh[:, l:h],
                                      scalar1=1.0 / 512.0)
                eng.tensor_tensor(out=t2h[:, l:h], in0=t2h[:, l:h],
                                  in1=t1h[:, l:h], op=OP.add)
            # y0 = 1.5 - vh ~= rsqrt(v+eps), y <- y*(1.5 - vh*y^2)
            eng.tensor_scalar(out=nsh, in0=t2h,
                              scalar1=-1.0, scalar2=1.5,
                              op0=OP.mult, op1=OP.add)
            for it in range(2):
                yout = rsh if it == 1 else nsh
                eng.tensor_tensor(out=t1h, in0=nsh, in1=nsh, op=OP.mult)
                eng.tensor_tensor(out=t1h, in0=t1h, in1=t2h, op=OP.mult)
                eng.tensor_scalar(out=t1h, in0=t1h,
                                  scalar1=-1.0, scalar2=1.5,
                                  op0=OP.mult, op1=OP.add)
                eng.tensor_tensor(out=yout, in0=nsh, in1=t1h, op=OP.mult)
            eng.tensor_tensor(out=nmh, in0=muh, in1=rsh, op=OP.mult)
            eng.tensor_scalar_mul(out=nmh, in0=nmh, scalar1=-1.0)

        emit_newton(0)

        # gelu (LN folded into per-partition scale/bias) + score dot.
        # sq7 (+ NewtonB) is emitted after g1: NewtonB only gates gelu4+,
        # and putting the square before g0 in ACT's in-order queue would
        # delay the whole gelu block by its 544ns.
        for t in range(ST):
            if t == 2:
                for u in range(4, ST):
                    if u in SQ_TILES:
                        emit_sq(u)
                emit_newton(1)
            hf, tt = t // HALF, t % HALF
            g_t = gelu_p.tile([P, D], f32, tag="gelu")
            if fast_ln:
                nc.scalar.activation(out=g_t, in_=ph_t(t),
                                     func=AF.Gelu,
                                     scale=rstd[hf][:, tt:tt + 1],
                                     bias=nmr[hf][:, tt:tt + 1])
            else:
                xh = gelu_p.tile([P, D], f32, tag="xh")
                nc.vector.tensor_scalar(out=xh, in0=ph_t(t),
                                        scalar1=mu[hf][:, tt:tt + 1],
                                        scalar2=rstd[hf][:, tt:tt + 1],
                                        op0=OP.subtract, op1=OP.mult)
                nc.vector.scalar_tensor_tensor(out=xh, in0=xh, scalar=1.0,
                                               in1=gm_sb, op0=OP.mult,
                                               op1=OP.mult)
                nc.vector.tensor_tensor(out=xh, in0=xh, in1=bt_sb,
                                        op=OP.add)
                nc.scalar.activation(out=g_t, in_=xh, func=AF.Gelu)
            # accum_out is a DVE-only feature (Pool's Q7 lacks the
            # accumulator), so every score dot rides DVE
            sc = scr_p.tile([P, D], f32, tag="scr")
            nc.vector.scalar_tensor_tensor(out=sc, in0=g_t, scalar=1.0,
                                           in1=w2_sb, op0=OP.bypass,
                                           op1=OP.mult,
                                           accum_out=s_col[hf][:, tt:tt + 1])

        # Tiny "poke" matmuls keep the PE clock ramped across the gelu/
        # score phase (a long idle resets the p-state and the pooled
        # chain then runs at 213-394ns/matmul instead of ~107).  Each is
        # gated on a progressively later small tensor so the greedy
        # scheduler cannot slot them before real matmuls, and each is
        # ~13ns so they never delay the pooled chain.  Results land in po
        # rows 0:8, fully overwritten by the real num chain.
        wsrc = xt_sb.bitcast(f32)

        def poke(psrc):
            nc.tensor.matmul(po[0:2, 0:8], lhsT=psrc,
                             rhs=wsrc[:, 1, 0:8],
                             start=True, stop=True, skip_group_check=True)

        for psrc in [mv_h[1][:, NBN - 1, 0:2], s_col[0][:, 0:2], s_col[0][:, 2:4],
                     s_col[1][:, 0:2]]:
            poke(psrc)

        # e^s = (e^(s/8))^8 with a degree-7 Taylor Horner chain entirely
        # on Pool ([P,4] ops ~3ns): |s/8| <= ~1 so rel err ~2e-5, and it
        # removes both ACT tanh ops and the DVE reciprocal chain from the
        # critical score7 -> mts -> pooled-matmul tail.
        import math
        HC = [1.0 / (math.factorial(k) * 8.0 ** k) for k in range(8)]
        mts = [big.tile([P, HALF, N], f32r, name=f"mts{h}", tag=f"mts{h}")
               for h in range(2)]
        for half in range(2):
            p = e_col[half]
            nc.gpsimd.tensor_scalar(out=p, in0=s_col[half],
                                    scalar1=HC[7], scalar2=HC[6],
                                    op0=OP.mult, op1=OP.add)
            for k in range(5, -1, -1):
                nc.gpsimd.tensor_tensor(out=p, in0=p, in1=s_col[half],
                                        op=OP.mult)
                nc.gpsimd.tensor_scalar_add(out=p, in0=p, scalar1=HC[k])
            for _ in range(3):  # ^8
                nc.gpsimd.tensor_tensor(out=p, in0=p, in1=p, op=OP.mult)
            if half == 0:
                poke(e_col[0][:, 0:2])
            # all mask scaling on Pool: DVE's in-order queue is busy with
            # score dots until ~sc7, which would delay mts (and the pooled
            # matmul chain) ~1.7us past data-readiness
            for tt in range(HALF):
                t = 4 * half + tt
                nc.gpsimd.tensor_scalar_mul(out=mts[half][:, tt, :],
                                            in0=mt_sb[:, t, :],
                                            scalar1=e_col[half][:, tt:tt + 1])
            # pooled num + den for this half's tiles; den rides its own
            # chain whose tile-t matmul precedes the num one, so den
            # completes first and the dinv chain overlaps the last num mm
            for tt in range(HALF):
                t = 4 * half + tt
                nc.tensor.matmul(pd[:, 0:2], lhsT=mts[half][:, tt, :],
                                 rhs=ones_r,
                                 start=(t == 0), stop=(t == ST - 1))
                nc.tensor.matmul(po[:, 0:D], lhsT=mts[half][:, tt, :],
                                 rhs=xp_sb[:, t, 0:D],
                                 start=(t == 0), stop=(t == ST - 1))

        # out = num * 1/(den + tiny); normalize split across ACT
        # (activation Copy with per-partition scale) and DVE, each half
        # DMA'd from its own queue so the DGE latencies overlap
        dinv = consts.tile([P, 1], f32)
        nc.vector.tensor_scalar_add(out=dinv, in0=pd[:, 0:1],
                                    scalar1=1e-30)
        nc.vector.reciprocal(out=dinv, in_=dinv)
        # single [P,256] normalize + single DMA: a split pair serializes
        # anyway (both halves read the one po tile, and the second DMA
        # pays the +630ns HWDGE stagger)
        out_sb = big.tile([P, D], f32, name="out_sb")
        nc.vector.tensor_scalar_mul(out=out_sb, in0=po[:, 0:D],
                                    scalar1=dinv)
        nc.sync.dma_start(out=out[:, :], in_=out_sb)

    nc.compile()
    _check_wait_counts(nc)
    return nc


def _check_wait_counts(nc):
    """TRN2 allows one sync wait per instruction (two on InstEventSemaphore);
    Bacc's generate_event_semaphores should guarantee this -- verify."""
    import json

    m = json.loads(nc.to_json_bytes())
    bad = []
    for f in m["functions"]:
        for blk in f["blocks"]:
            for ins in blk["instructions"]:
                op = str(ins.get("opcode", ""))
                waits = (ins.get("sync_info") or {}).get("on_wait") or []
                limit = 2 if ("EventSemaphore" in op or "Drain" in op) else 1
                if len(waits) > limit:
                    bad.append((ins.get("name"), op,
                                [(w.get("ant_name"), w.get("wait_value"))
                                 for w in waits]))
    if bad:
        raise AssertionError(f"instructions over the wait limit: {bad}")


def kernel(doc_state, nodes_mapping, nodes_len, W1, b1, gamma, beta, W2, b2,
           _trace=False):
    from concourse.bass_utils import run_bass_kernel_spmd

    doc_state = np.ascontiguousarray(doc_state, dtype=np.float32)
    nodes_mapping = np.asarray(nodes_mapping, dtype=np.float32)
    W1 = np.asarray(W1, dtype=np.float32)
    W2v = np.asarray(W2, np.float32).reshape(D)
    b1 = np.asarray(b1, dtype=np.float32).reshape(-1)
    gamma = np.asarray(gamma, dtype=np.float32).reshape(-1)
    beta = np.asarray(beta, dtype=np.float32).reshape(-1)

    fast_ln = (not b1.any()) and bool(np.all(gamma == 1.0)) and (not beta.any())
    key = ("nc", fast_ln)
    if key not in _CACHE:
        _CACHE[key] = _build(fast_ln)
    nc = _CACHE[key]

    # [W1 c0 | mean col, W1 c1 | mean col, W2 row | mean(b1)]
    w1m = np.zeros((P, 3, D + 2), np.float32)
    for c in range(DC):
        w1m[:, c, 0:D] = W1[c * P:(c + 1) * P]
        w1m[:, c, D] = W1[c * P:(c + 1) * P].sum(axis=1) / np.float32(D)
    w1m[:, 2, 0:D] = W2v[None, :]
    w1m[:, 2, D] = np.float32(b1.mean() if b1.size else 0.0)
    w1m = np.ascontiguousarray(w1m)

    # host-side input prep: transposed X for the scorer, ones-padded X for
    # the fused num|den pooled matmul, transposed binary mask
    xt_all = np.ascontiguousarray(doc_state.transpose(0, 2, 1))
    xp_all = np.empty((B, S, D + 2), np.float32)
    xp_all[:, :, 0:D] = doc_state
    xp_all[:, :, D:] = 1.0
    mt_all = np.ascontiguousarray(nodes_mapping.transpose(0, 2, 1))

    in_maps = []
    for b in range(B):
        m = {"xt": xt_all[b], "xp": xp_all[b], "mt": mt_all[b], "w1m": w1m}
        if not fast_ln:
            m["b1"] = b1.reshape(1, D)
            m["gamma"] = gamma.reshape(1, D)
            m["beta"] = beta.reshape(1, D)
        in_maps.append(m)

    res = run_bass_kernel_spmd(nc, in_maps, core_ids=list(range(B)),
                               trace=_trace)
    out = np.stack([res.results[b]["out"] for b in range(B)], axis=0)
    if _trace:
        kernel.last_exec_time_ns = res.exec_time_ns
        kernel.last_trace = res.instructions_and_trace
    return out
